# revision 22
# baseline (speedup 1.0000x reference)
"""CrossAttention kernel for 8 TRN2 NeuronCores (data-parallel over batch).

fp8(e4m3) + DoubleRow version. Per batch element b (one core each):
  q32 = (32*Wq)_fp8 @ x_fp8            # [512, 4096] psum, DoubleRow K=256
  kT32 = (32*Wk)_fp8 @ ctx_fp8.T       # [512, 256]  psum, DoubleRow
  v = ((ctx_fp8 @ (32*Wv)_fp8.T)/32)   # [256, 512] -> fp8
  per head h:
    simT32 = kT32_h.T @ q32_h          # bf16 matmul (sim is output-rate bound)
    E = exp(simT32 * SCALE/1024) fp8   # descale folded into ACT scale
    av = v_h.T @ E                     # DoubleRow K=256, one E pass
    S  = ones.T @ E                    # DoubleRow, S replicated over 64 rows
    oc_h = (av / S) fp8
  o32 = (32*Wout)_fp8 @ oc + 32*(x+bout)   # bf16 out; host divides by 32

All projections use fp8 DoubleRow (2 fp8 MACs/cell/cycle). Weights are
quantized x32 on host to stay clear of e4m3 subnormals; descales are folded
into existing casts (free) and into the final host-side /32.
"""

import numpy as np
import ml_dtypes

import concourse.bass as bass
import concourse.mybir as mybir
import concourse.tile as tile
from concourse import bacc
from concourse.bass_utils import run_bass_kernel_spmd

HEADS = 8
DIM_HEAD = 64
SCALE = DIM_HEAD ** -0.5
DIM = 512          # channels of x
CTX_DIM = 768
N_CTX = 256        # context positions
HW = 4096          # 64*64 pixels
CH = 512           # i-chunk size
NCHUNK = HW // CH  # 8
B = 8              # batch == number of cores

F32 = mybir.dt.float32
BF16 = mybir.dt.bfloat16
F8 = mybir.dt.float8e4
DR = mybir.MatmulPerfMode.DoubleRow


def build_bass(loop_n=1):
    nc = bacc.Bacc(
        "TRN2",
        target_bir_lowering=False,
        debug=False,
        num_devices=B,
    )

    # DRAM parameters, host-pre-shuffled to partition-major contiguous layouts
    # so every DMA is one contiguous descriptor per partition.
    x8_d = nc.declare_dram_parameter("x8", [128, NCHUNK, 4, CH], F8, isOutput=False)
    xres_d = nc.declare_dram_parameter("xres32", [128, NCHUNK, 4, CH], BF16, isOutput=False)
    ctxT_d = nc.declare_dram_parameter("ctxT8", [128, 6, N_CTX], F8, isOutput=False)
    wqT_d = nc.declare_dram_parameter("wqT8", [128, 4, DIM], F8, isOutput=False)
    wkT_d = nc.declare_dram_parameter("wkT8", [128, 6, DIM], F8, isOutput=False)
    wvT_d = nc.declare_dram_parameter("wvT8", [128, 6, DIM], F8, isOutput=False)
    woutT_d = nc.declare_dram_parameter("woT8", [128, 4, DIM], F8, isOutput=False)
    out_d = nc.declare_dram_parameter("out", [128, NCHUNK, 4, CH], BF16, isOutput=True)

    x8_t = x8_d[:]          # [128, 8, 4, 512]
    xres_t = xres_d[:]
    ctxT_t = ctxT_d[:]      # [128, 6, 256]
    wqT_t = wqT_d[:]        # [128, 4, 512]
    wkT_t = wkT_d[:]
    wvT_t = wvT_d[:]
    woutT_t = woutT_d[:]
    out_t = out_d[:]        # [128, 8, 4, 512]

    with tile.TileContext(nc) as tc:
        with (
            tc.tile_pool(name="wts", bufs=1) as wts,
            tc.tile_pool(name="kv", bufs=1) as kvp,
            tc.tile_pool(name="xp", bufs=3) as xp,
            tc.tile_pool(name="qp", bufs=2) as qp,
            tc.tile_pool(name="ep", bufs=3) as ep,
            tc.tile_pool(name="rp", bufs=3) as rp,
            tc.tile_pool(name="ocp", bufs=2) as ocp,
            tc.tile_pool(name="outp", bufs=2) as outp,
            tc.tile_pool(name="psav", bufs=2, space="PSUM") as psav,
            tc.tile_pool(name="psio", bufs=2, space="PSUM") as ps,
            tc.tile_pool(name="ps2", bufs=2, space="PSUM") as ps2,
        ):
            # ---- load weights / context (kT deps first, desc-gen in parallel
            # across engine queues to cut startup latency) ----
            wk_sb = wts.tile([128, 6, DIM], F8)
            nc.gpsimd.dma_start(out=wk_sb, in_=wkT_t)
            ctx_sb = wts.tile([128, 6, N_CTX], F8)
            nc.sync.dma_start(out=ctx_sb, in_=ctxT_t)
            wv_sb = wts.tile([128, 6, DIM], F8)
            nc.scalar.dma_start(out=wv_sb, in_=wvT_t)
            wq_sb = wts.tile([128, 4, DIM], F8)
            nc.sync.dma_start(out=wq_sb, in_=wqT_t)
            wo_sb = wts.tile([128, 4, DIM], F8)
            nc.gpsimd.dma_start(out=wo_sb, in_=woutT_t)
            ones_sb = wts.tile([128, DIM_HEAD], BF16)
            nc.vector.memset(ones_sb, 1.0)

            # loop_n > 1 repeats the whole compute for slope-based timing
            for _it in range(loop_n):
                # ---- kT32 = (32WkT).T @ ctxT8 : [512, 256] as [128, 4, 256] bf16
                kT_sb = kvp.tile([128, 4, N_CTX], BF16, tag="kT")
                for m in range(4):
                    pt = ps.tile([128, CH], F32, tag="ps")
                    for u in range(3):
                        nc.tensor.matmul(
                            pt[:, :N_CTX],
                            wk_sb[:, 2 * u:2 * u + 2, bass.ts(m, 128)],
                            ctx_sb[:, 2 * u:2 * u + 2, :],
                            start=(u == 0),
                            stop=(u == 2),
                            perf_mode=DR,
                        )
                    nc.scalar.copy(out=kT_sb[:, m, :], in_=pt[:, :N_CTX])

                # ---- v = (ctxT8.T @ 32WvT)/32 : [256, 512] bf16 as [128, 2, 512]
                v_sb = kvp.tile([128, 2, DIM], BF16, tag="v")
                for m in range(2):
                    pt = ps.tile([128, CH], F32, tag="ps")
                    for u in range(3):
                        nc.tensor.matmul(
                            pt,
                            ctx_sb[:, 2 * u:2 * u + 2, bass.ts(m, 128)],
                            wv_sb[:, 2 * u:2 * u + 2, :],
                            start=(u == 0),
                            stop=(u == 2),
                            perf_mode=DR,
                        )
                    nc.scalar.mul(out=v_sb[:, m, :], in_=pt, mul=1.0 / 32)

                # ---- main loop over pixel chunks ----
                for c in range(NCHUNK):
                    x8_sb = xp.tile([128, 4, CH], F8, tag="x8")
                    nc.gpsimd.dma_start(out=x8_sb, in_=x8_t[:, c, :, :])
                    xr_sb = xp.tile([128, 4, CH], BF16, tag="xr")
                    nc.gpsimd.dma_start(out=xr_sb, in_=xres_t[:, c, :, :])

                    # q32 = (32WqT).T @ x8 -> [128, 4, CH] bf16 (carries x32)
                    q_sb = qp.tile([128, 4, CH], BF16)
                    for m in range(4):
                        pt = ps.tile([128, CH], F32, tag="ps")
                        for u in range(2):
                            nc.tensor.matmul(
                                pt,
                                wq_sb[:, 2 * u:2 * u + 2, bass.ts(m, 128)],
                                x8_sb[:, 2 * u:2 * u + 2, :],
                                start=(u == 0),
                                stop=(u == 1),
                                perf_mode=DR,
                            )
                        nc.vector.tensor_copy(out=q_sb[:, m, :], in_=pt)

                    # per head-pair attention
                    oc_sb = ocp.tile([128, 4, CH], F8)
                    for p in range(4):  # head pair p -> heads 2p, 2p+1
                        # simT (x1024) for both heads, bf16 matmuls
                        pts = [ps2.tile([128, 2, CH], F32, tag="sim",
                                        name=f"psim{p}_{hh2}")
                               for hh2 in range(2)]
                        for hh in range(2):
                            for j in range(2):
                                h0 = hh * 64
                                nc.tensor.matmul(
                                    pts[hh][:, j, :],
                                    kT_sb[h0:h0 + 64, p, bass.ts(j, 128)],
                                    q_sb[h0:h0 + 64, p, :],
                                    start=True,
                                    stop=True,
                                )
                        # E = exp(simT32 * SCALE/1024) -> bf16, one ACT op/head
                        e_tiles = []
                        for hh in range(2):
                            e_sb = ep.tile([128, 2, CH], BF16, tag="e")
                            nc.scalar.activation(
                                out=e_sb,
                                in_=pts[hh],
                                func=mybir.ActivationFunctionType.Exp,
                                scale=float(SCALE) / 1024.0,
                            )
                            e_tiles.append(e_sb)

                        # attn@v + denominator, col-group pairs (bf16)
                        pav = psav.tile([128, CH], F32, tag="pav")
                        pS = psav.tile([128, CH], F32, tag="pav", name=f"pS{p}")
                        for kj in range(2):
                            for hh in range(2):
                                h = 2 * p + hh
                                h0 = hh * 64
                                nc.tensor.matmul(
                                    pav[h0:h0 + 64, :],
                                    v_sb[:, kj, bass.ds(h * 64, 64)],
                                    e_tiles[hh][:, kj, :],
                                    start=(kj == 0),
                                    stop=(kj == 1),
                                    skip_group_check=True,
                                )
                        for kj in range(2):
                            for hh in range(2):
                                h0 = hh * 64
                                nc.tensor.matmul(
                                    pS[h0:h0 + 64, :],
                                    ones_sb,
                                    e_tiles[hh][:, kj, :],
                                    start=(kj == 0),
                                    stop=(kj == 1),
                                    skip_group_check=True,
                                )
                        # normalize: oc = pav / pS  (full 128-width), fp8 out
                        r_sb = rp.tile([128, CH], F32, tag="r")
                        nc.vector.reciprocal_approx_fast(out=r_sb, in_=pS)
                        nc.vector.tensor_mul(out=oc_sb[:, p, :], in0=pav, in1=r_sb)

                    # out projection (x32) + residual 32*(x+bout), bf16 out
                    o_sb = outp.tile([128, 4, CH], BF16)
                    for m in range(4):
                        pt = ps.tile([128, CH], F32, tag="ps")
                        for u in range(2):
                            nc.tensor.matmul(
                                pt,
                                wo_sb[:, 2 * u:2 * u + 2, bass.ts(m, 128)],
                                oc_sb[:, 2 * u:2 * u + 2, :],
                                start=(u == 0),
                                stop=(u == 1),
                                perf_mode=DR,
                            )
                        nc.vector.tensor_add(
                            out=o_sb[:, m, :],
                            in0=pt,
                            in1=xr_sb[:, m, :],
                        )
                    nc.gpsimd.dma_start(out=out_t[:, c, :, :], in_=o_sb)

    nc.compile()
    return nc


_NC_CACHE = None


def _get_nc():
    global _NC_CACHE
    if _NC_CACHE is None:
        _NC_CACHE = build_bass()
    return _NC_CACHE


def _pshuf_w(a):
    """[T*128, E] -> [128, T, E] partition-major contiguous."""
    t = a.shape[0] // 128
    return np.ascontiguousarray(a.reshape(t, 128, a.shape[1]).transpose(1, 0, 2))


def _pshuf_x(a):
    """[512, 4096] -> [128, NCHUNK, 4, CH] partition-major, chunk-contiguous."""
    return np.ascontiguousarray(
        a.reshape(4, 128, NCHUNK, CH).transpose(1, 2, 0, 3))


def make_in_maps(x, context, Wq, Wkv, Wout, bout):
    """Host-side prep: shard over batch, pre-transpose weights, quantize fp8."""
    f = np.float32
    bf = ml_dtypes.bfloat16
    f8 = ml_dtypes.float8_e4m3
    wqT = _pshuf_w((Wq.T * np.float32(32)).astype(f8))
    wkT = _pshuf_w((Wkv[:512].T * np.float32(32)).astype(f8))
    wvT = _pshuf_w((Wkv[512:].T * np.float32(32)).astype(f8))
    woT = _pshuf_w((Wout.T * np.float32(32)).astype(f8))
    bout = np.asarray(bout, dtype=f)
    in_maps = []
    for b in range(B):
        xf = np.ascontiguousarray(x[b].reshape(DIM, HW), dtype=f)
        in_maps.append({
            "x8": _pshuf_x(xf.astype(f8)),
            "xres32": _pshuf_x(((xf + bout[:, None]) * np.float32(32)).astype(bf)),
            "ctxT8": _pshuf_w(np.ascontiguousarray(context[b].T).astype(f8)),
            "wqT8": wqT,
            "wkT8": wkT,
            "wvT8": wvT,
            "woT8": woT,
        })
    return in_maps


def postprocess_out(raw):
    """Device out [128, NCHUNK, 4, CH] bf16 carrying x32 -> [512, 4096] f32."""
    o = np.asarray(raw).astype(np.float32).reshape(128, NCHUNK, 4, CH)
    return (o / np.float32(32)).transpose(2, 0, 1, 3).reshape(DIM, HW)


def kernel(x, context, Wq, Wkv, Wout, bout):
    x = np.asarray(x)
    context = np.asarray(context)
    nc = _get_nc()
    in_maps = make_in_maps(x, context, np.asarray(Wq), np.asarray(Wkv),
                           np.asarray(Wout), np.asarray(bout))
    res = run_bass_kernel_spmd(nc, in_maps, core_ids=list(range(B)))
    outs = [postprocess_out(res.results[b]["out"]) for b in range(B)]
    return np.stack(outs, axis=0).reshape(B, DIM, 64, 64)


# revision 24
# speedup vs baseline: 1.1165x; 1.1165x over previous
"""CrossAttention kernel for 8 TRN2 NeuronCores (data-parallel over batch).

fp8(e4m3) + DoubleRow version. Per batch element b (one core each):
  q32 = (32*Wq)_fp8 @ x_fp8            # [512, 4096] psum, DoubleRow K=256
  kT32 = (32*Wk)_fp8 @ ctx_fp8.T       # [512, 256]  psum, DoubleRow
  v = ((ctx_fp8 @ (32*Wv)_fp8.T)/32)   # [256, 512] -> fp8
  per head h:
    simT32 = kT32_h.T @ q32_h          # bf16 matmul (sim is output-rate bound)
    E = exp(simT32 * SCALE/1024) fp8   # descale folded into ACT scale
    av = v_h.T @ E                     # DoubleRow K=256, one E pass
    S  = ones.T @ E                    # DoubleRow, S replicated over 64 rows
    oc_h = (av / S) fp8
  o32 = (32*Wout)_fp8 @ oc + 32*(x+bout)   # bf16 out; host divides by 32

All projections use fp8 DoubleRow (2 fp8 MACs/cell/cycle). Weights are
quantized x32 on host to stay clear of e4m3 subnormals; descales are folded
into existing casts (free) and into the final host-side /32.
"""

import numpy as np
import ml_dtypes

import concourse.bass as bass
import concourse.mybir as mybir
import concourse.tile as tile
from concourse import bacc
from concourse.bass_utils import run_bass_kernel_spmd

HEADS = 8
DIM_HEAD = 64
SCALE = DIM_HEAD ** -0.5
DIM = 512          # channels of x
CTX_DIM = 768
N_CTX = 256        # context positions
HW = 4096          # 64*64 pixels
CH = 512           # i-chunk size
NCHUNK = HW // CH  # 8
B = 8              # batch == number of cores

F32 = mybir.dt.float32
BF16 = mybir.dt.bfloat16
F8 = mybir.dt.float8e4
DR = mybir.MatmulPerfMode.DoubleRow


def build_bass(loop_n=1):
    nc = bacc.Bacc(
        "TRN2",
        target_bir_lowering=False,
        debug=False,
        num_devices=B,
    )

    # DRAM parameters, host-pre-shuffled to partition-major contiguous layouts
    # so every DMA is one contiguous descriptor per partition.
    x8_d = nc.declare_dram_parameter("x8", [128, NCHUNK, 4, CH], F8, isOutput=False)
    xres_d = nc.declare_dram_parameter("xres32", [128, NCHUNK, 4, CH], BF16, isOutput=False)
    ctxT_d = nc.declare_dram_parameter("ctxT8", [128, 6, N_CTX], F8, isOutput=False)
    wqT_d = nc.declare_dram_parameter("wqT8", [128, 4, DIM], F8, isOutput=False)
    wkT_d = nc.declare_dram_parameter("wkT8", [128, 6, DIM], F8, isOutput=False)
    wvT_d = nc.declare_dram_parameter("wvT8", [128, 6, DIM], F8, isOutput=False)
    woutT_d = nc.declare_dram_parameter("woT8", [128, 4, DIM], F8, isOutput=False)
    out_d = nc.declare_dram_parameter("out", [128, NCHUNK, 4, CH], BF16, isOutput=True)

    x8_t = x8_d[:]          # [128, 8, 4, 512]
    xres_t = xres_d[:]
    ctxT_t = ctxT_d[:]      # [128, 6, 256]
    wqT_t = wqT_d[:]        # [128, 4, 512]
    wkT_t = wkT_d[:]
    wvT_t = wvT_d[:]
    woutT_t = woutT_d[:]
    out_t = out_d[:]        # [128, 8, 4, 512]

    with tile.TileContext(nc) as tc:
        with (
            tc.tile_pool(name="wts", bufs=1) as wts,
            tc.tile_pool(name="kv", bufs=1) as kvp,
            tc.tile_pool(name="xp", bufs=3) as xp,
            tc.tile_pool(name="qp", bufs=2) as qp,
            tc.tile_pool(name="ep", bufs=3) as ep,
            tc.tile_pool(name="rp", bufs=3) as rp,
            tc.tile_pool(name="ocp", bufs=2) as ocp,
            tc.tile_pool(name="outp", bufs=2) as outp,
            tc.tile_pool(name="ps", bufs=2, space="PSUM") as ps,
            tc.tile_pool(name="ps2", bufs=3, space="PSUM") as ps2,
        ):
            # ---- load weights / context (kT deps first, desc-gen in parallel
            # across engine queues to cut startup latency) ----
            wk_sb = wts.tile([128, 6, DIM], F8)
            nc.gpsimd.dma_start(out=wk_sb, in_=wkT_t)
            ctx_sb = wts.tile([128, 6, N_CTX], F8)
            nc.sync.dma_start(out=ctx_sb, in_=ctxT_t)
            wv_sb = wts.tile([128, 6, DIM], F8)
            nc.scalar.dma_start(out=wv_sb, in_=wvT_t)
            wq_sb = wts.tile([128, 4, DIM], F8)
            nc.sync.dma_start(out=wq_sb, in_=wqT_t)
            wo_sb = wts.tile([128, 4, DIM], F8)
            nc.gpsimd.dma_start(out=wo_sb, in_=woutT_t)
            ones_sb = wts.tile([128, DIM_HEAD], BF16)
            nc.vector.memset(ones_sb, 1.0)

            # loop_n > 1 repeats the whole compute for slope-based timing
            for _it in range(loop_n):
                # ---- kT32 = (32WkT).T @ ctxT8 : [512, 256] as [128, 4, 256] bf16
                kT_sb = kvp.tile([128, 4, N_CTX], BF16, tag="kT")
                for m in range(4):
                    pt = ps.tile([128, CH], F32, tag="ps")
                    for u in range(3):
                        nc.tensor.matmul(
                            pt[:, :N_CTX],
                            wk_sb[:, 2 * u:2 * u + 2, bass.ts(m, 128)],
                            ctx_sb[:, 2 * u:2 * u + 2, :],
                            start=(u == 0),
                            stop=(u == 2),
                            perf_mode=DR,
                        )
                    nc.scalar.copy(out=kT_sb[:, m, :], in_=pt[:, :N_CTX])

                # ---- v = (ctxT8.T @ 32WvT)/32 : [256, 512] bf16 as [128, 2, 512]
                v_sb = kvp.tile([128, 2, DIM], BF16, tag="v")
                for m in range(2):
                    pt = ps.tile([128, CH], F32, tag="ps")
                    for u in range(3):
                        nc.tensor.matmul(
                            pt,
                            ctx_sb[:, 2 * u:2 * u + 2, bass.ts(m, 128)],
                            wv_sb[:, 2 * u:2 * u + 2, :],
                            start=(u == 0),
                            stop=(u == 2),
                            perf_mode=DR,
                        )
                    nc.scalar.mul(out=v_sb[:, m, :], in_=pt, mul=1.0 / 32)

                # ---- main loop over pixel chunks ----
                for c in range(NCHUNK):
                    x8_sb = xp.tile([128, 4, CH], F8, tag="x8")
                    nc.gpsimd.dma_start(out=x8_sb, in_=x8_t[:, c, :, :])
                    xr_sb = xp.tile([128, 4, CH], BF16, tag="xr")
                    nc.gpsimd.dma_start(out=xr_sb, in_=xres_t[:, c, :, :])

                    # q32 = (32WqT).T @ x8 -> [128, 4, CH] bf16 (carries x32)
                    q_sb = qp.tile([128, 4, CH], BF16)
                    for m in range(4):
                        pt = ps.tile([128, CH], F32, tag="ps")
                        for u in range(2):
                            nc.tensor.matmul(
                                pt,
                                wq_sb[:, 2 * u:2 * u + 2, bass.ts(m, 128)],
                                x8_sb[:, 2 * u:2 * u + 2, :],
                                start=(u == 0),
                                stop=(u == 1),
                                perf_mode=DR,
                            )
                        nc.vector.tensor_copy(out=q_sb[:, m, :], in_=pt)

                    # per head-pair attention
                    oc_sb = ocp.tile([128, 4, CH], F8)
                    for p in range(4):  # head pair p -> heads 2p, 2p+1
                        # simT (x1024) for both heads, bf16 matmuls
                        pts = [ps2.tile([128, 2, CH], F32, tag="sim",
                                        name=f"psim{p}_{hh2}")
                               for hh2 in range(2)]
                        for hh in range(2):
                            for j in range(2):
                                h0 = hh * 64
                                nc.tensor.matmul(
                                    pts[hh][:, j, :],
                                    kT_sb[h0:h0 + 64, p, bass.ts(j, 128)],
                                    q_sb[h0:h0 + 64, p, :],
                                    start=True,
                                    stop=True,
                                )
                        # E = exp(simT32 * SCALE/1024) -> bf16, one ACT op/head
                        e_tiles = []
                        for hh in range(2):
                            e_sb = ep.tile([128, 2, CH], BF16, tag="e")
                            nc.scalar.activation(
                                out=e_sb,
                                in_=pts[hh],
                                func=mybir.ActivationFunctionType.Exp,
                                scale=float(SCALE) / 1024.0,
                            )
                            e_tiles.append(e_sb)

                        # attn@v + denominator, col-group pairs (bf16)
                        pav = ps.tile([128, CH], F32, tag="ps")
                        pS = ps.tile([128, CH], F32, tag="ps", name=f"pS{p}")
                        for kj in range(2):
                            for hh in range(2):
                                h = 2 * p + hh
                                h0 = hh * 64
                                nc.tensor.matmul(
                                    pav[h0:h0 + 64, :],
                                    v_sb[:, kj, bass.ds(h * 64, 64)],
                                    e_tiles[hh][:, kj, :],
                                    start=(kj == 0),
                                    stop=(kj == 1),
                                    skip_group_check=True,
                                )
                        for kj in range(2):
                            for hh in range(2):
                                h0 = hh * 64
                                nc.tensor.matmul(
                                    pS[h0:h0 + 64, :],
                                    ones_sb,
                                    e_tiles[hh][:, kj, :],
                                    start=(kj == 0),
                                    stop=(kj == 1),
                                    skip_group_check=True,
                                )
                        # normalize: oc = pav / pS  (full 128-width), fp8 out
                        r_sb = rp.tile([128, CH], F32, tag="r")
                        nc.vector.reciprocal_approx_fast(out=r_sb, in_=pS)
                        nc.vector.tensor_mul(out=oc_sb[:, p, :], in0=pav, in1=r_sb)

                    # out projection (x32) + residual 32*(x+bout), bf16 out
                    o_sb = outp.tile([128, 4, CH], BF16)
                    for m in range(4):
                        pt = ps.tile([128, CH], F32, tag="ps")
                        for u in range(2):
                            nc.tensor.matmul(
                                pt,
                                wo_sb[:, 2 * u:2 * u + 2, bass.ts(m, 128)],
                                oc_sb[:, 2 * u:2 * u + 2, :],
                                start=(u == 0),
                                stop=(u == 1),
                                perf_mode=DR,
                            )
                        nc.vector.tensor_add(
                            out=o_sb[:, m, :],
                            in0=pt,
                            in1=xr_sb[:, m, :],
                        )
                    nc.gpsimd.dma_start(out=out_t[:, c, :, :], in_=o_sb)

    nc.compile()
    return nc


_NC_CACHE = None


def _get_nc():
    global _NC_CACHE
    if _NC_CACHE is None:
        _NC_CACHE = build_bass()
    return _NC_CACHE


def _pshuf_w(a):
    """[T*128, E] -> [128, T, E] partition-major contiguous."""
    t = a.shape[0] // 128
    return np.ascontiguousarray(a.reshape(t, 128, a.shape[1]).transpose(1, 0, 2))


def _pshuf_x(a):
    """[512, 4096] -> [128, NCHUNK, 4, CH] partition-major, chunk-contiguous."""
    return np.ascontiguousarray(
        a.reshape(4, 128, NCHUNK, CH).transpose(1, 2, 0, 3))


def make_in_maps(x, context, Wq, Wkv, Wout, bout):
    """Host-side prep: shard over batch, pre-transpose weights, quantize fp8."""
    f = np.float32
    bf = ml_dtypes.bfloat16
    f8 = ml_dtypes.float8_e4m3
    wqT = _pshuf_w((Wq.T * np.float32(32)).astype(f8))
    wkT = _pshuf_w((Wkv[:512].T * np.float32(32)).astype(f8))
    wvT = _pshuf_w((Wkv[512:].T * np.float32(32)).astype(f8))
    woT = _pshuf_w((Wout.T * np.float32(32)).astype(f8))
    bout = np.asarray(bout, dtype=f)
    in_maps = []
    for b in range(B):
        xf = np.ascontiguousarray(x[b].reshape(DIM, HW), dtype=f)
        in_maps.append({
            "x8": _pshuf_x(xf.astype(f8)),
            "xres32": _pshuf_x(((xf + bout[:, None]) * np.float32(32)).astype(bf)),
            "ctxT8": _pshuf_w(np.ascontiguousarray(context[b].T).astype(f8)),
            "wqT8": wqT,
            "wkT8": wkT,
            "wvT8": wvT,
            "woT8": woT,
        })
    return in_maps


def postprocess_out(raw):
    """Device out [128, NCHUNK, 4, CH] bf16 carrying x32 -> [512, 4096] f32."""
    o = np.asarray(raw).astype(np.float32).reshape(128, NCHUNK, 4, CH)
    return (o / np.float32(32)).transpose(2, 0, 1, 3).reshape(DIM, HW)


def kernel(x, context, Wq, Wkv, Wout, bout):
    x = np.asarray(x)
    context = np.asarray(context)
    nc = _get_nc()
    in_maps = make_in_maps(x, context, np.asarray(Wq), np.asarray(Wkv),
                           np.asarray(Wout), np.asarray(bout))
    res = run_bass_kernel_spmd(nc, in_maps, core_ids=list(range(B)))
    outs = [postprocess_out(res.results[b]["out"]) for b in range(B)]
    return np.stack(outs, axis=0).reshape(B, DIM, 64, 64)


# revision 26
# speedup vs baseline: 1.1773x; 1.0545x over previous
"""CrossAttention kernel for 8 TRN2 NeuronCores (data-parallel over batch).

fp8(e4m3) + DoubleRow version. Per batch element b (one core each):
  q32 = (32*Wq)_fp8 @ x_fp8            # [512, 4096] psum, DoubleRow K=256
  kT32 = (32*Wk)_fp8 @ ctx_fp8.T       # [512, 256]  psum, DoubleRow
  v = ((ctx_fp8 @ (32*Wv)_fp8.T)/32)   # [256, 512] -> fp8
  per head h:
    simT32 = kT32_h.T @ q32_h          # bf16 matmul (sim is output-rate bound)
    E = exp(simT32 * SCALE/1024) fp8   # descale folded into ACT scale
    av = v_h.T @ E                     # DoubleRow K=256, one E pass
    S  = ones.T @ E                    # DoubleRow, S replicated over 64 rows
    oc_h = (av / S) fp8
  o32 = (32*Wout)_fp8 @ oc + 32*(x+bout)   # bf16 out; host divides by 32

All projections use fp8 DoubleRow (2 fp8 MACs/cell/cycle). Weights are
quantized x32 on host to stay clear of e4m3 subnormals; descales are folded
into existing casts (free) and into the final host-side /32.
"""

import numpy as np
import ml_dtypes

import concourse.bass as bass
import concourse.mybir as mybir
import concourse.tile as tile
from concourse import bacc
from concourse.bass_utils import run_bass_kernel_spmd

HEADS = 8
DIM_HEAD = 64
SCALE = DIM_HEAD ** -0.5
DIM = 512          # channels of x
CTX_DIM = 768
N_CTX = 256        # context positions
HW = 4096          # 64*64 pixels
CH = 512           # i-chunk size
NCHUNK = HW // CH  # 8
B = 8              # batch == number of cores

F32 = mybir.dt.float32
BF16 = mybir.dt.bfloat16
F8 = mybir.dt.float8e4
DR = mybir.MatmulPerfMode.DoubleRow


def build_bass(loop_n=1):
    nc = bacc.Bacc(
        "TRN2",
        target_bir_lowering=False,
        debug=False,
        num_devices=B,
    )

    # DRAM parameters, host-pre-shuffled to partition-major contiguous layouts
    # so every DMA is one contiguous descriptor per partition.
    x8_d = nc.declare_dram_parameter("x8", [128, NCHUNK, 4, CH], F8, isOutput=False)
    xres_d = nc.declare_dram_parameter("xres32", [128, NCHUNK, 4, CH], BF16, isOutput=False)
    ctxT_d = nc.declare_dram_parameter("ctxT8", [128, 6, N_CTX], F8, isOutput=False)
    wqT_d = nc.declare_dram_parameter("wqT8", [128, 4, DIM], F8, isOutput=False)
    wkT_d = nc.declare_dram_parameter("wkT8", [128, 6, DIM], F8, isOutput=False)
    wvT_d = nc.declare_dram_parameter("wvT8", [128, 6, DIM], F8, isOutput=False)
    woutT_d = nc.declare_dram_parameter("woT8", [128, 4, DIM], F8, isOutput=False)
    out_d = nc.declare_dram_parameter("out", [128, NCHUNK, 4, CH], BF16, isOutput=True)

    x8_t = x8_d[:]          # [128, 8, 4, 512]
    xres_t = xres_d[:]
    ctxT_t = ctxT_d[:]      # [128, 6, 256]
    wqT_t = wqT_d[:]        # [128, 4, 512]
    wkT_t = wkT_d[:]
    wvT_t = wvT_d[:]
    woutT_t = woutT_d[:]
    out_t = out_d[:]        # [128, 8, 4, 512]

    with tile.TileContext(nc) as tc:
        with (
            tc.tile_pool(name="wts", bufs=1) as wts,
            tc.tile_pool(name="kv", bufs=1) as kvp,
            tc.tile_pool(name="xp", bufs=3) as xp,
            tc.tile_pool(name="qp", bufs=2) as qp,
            tc.tile_pool(name="ep", bufs=3) as ep,
            tc.tile_pool(name="rp", bufs=3) as rp,
            tc.tile_pool(name="ocp", bufs=2) as ocp,
            tc.tile_pool(name="outp", bufs=2) as outp,
            tc.tile_pool(name="ps", bufs=2, space="PSUM") as ps,
            tc.tile_pool(name="ps2", bufs=3, space="PSUM") as ps2,
        ):
            # ---- load weights / context (kT deps first, desc-gen in parallel
            # across engine queues to cut startup latency) ----
            wk_sb = wts.tile([128, 6, DIM], F8)
            nc.gpsimd.dma_start(out=wk_sb, in_=wkT_t)
            ctx_sb = wts.tile([128, 6, N_CTX], F8)
            nc.sync.dma_start(out=ctx_sb, in_=ctxT_t)
            wv_sb = wts.tile([128, 6, DIM], F8)
            nc.scalar.dma_start(out=wv_sb, in_=wvT_t)
            wq_sb = wts.tile([128, 4, DIM], F8)
            nc.sync.dma_start(out=wq_sb, in_=wqT_t)
            wo_sb = wts.tile([128, 4, DIM], F8)
            nc.gpsimd.dma_start(out=wo_sb, in_=woutT_t)
            ones_sb = wts.tile([128, DIM_HEAD], BF16)
            nc.vector.memset(ones_sb, 1.0)

            # loop_n > 1 repeats the whole compute for slope-based timing
            for _it in range(loop_n):
                # ---- kT32 = (32WkT).T @ ctxT8 : [512, 256] as [128, 4, 256] bf16
                kT_sb = kvp.tile([128, 4, N_CTX], BF16, tag="kT")
                for m in range(4):
                    pt = ps.tile([128, CH], F32, tag="ps")
                    for u in range(3):
                        nc.tensor.matmul(
                            pt[:, :N_CTX],
                            wk_sb[:, 2 * u:2 * u + 2, bass.ts(m, 128)],
                            ctx_sb[:, 2 * u:2 * u + 2, :],
                            start=(u == 0),
                            stop=(u == 2),
                            perf_mode=DR,
                        )
                    nc.scalar.copy(out=kT_sb[:, m, :], in_=pt[:, :N_CTX])

                # ---- v = (ctxT8.T @ 32WvT)/32 : [256, 512] bf16 as [128, 2, 512]
                v_sb = kvp.tile([128, 2, DIM], BF16, tag="v")
                for m in range(2):
                    pt = ps.tile([128, CH], F32, tag="ps")
                    for u in range(3):
                        nc.tensor.matmul(
                            pt,
                            ctx_sb[:, 2 * u:2 * u + 2, bass.ts(m, 128)],
                            wv_sb[:, 2 * u:2 * u + 2, :],
                            start=(u == 0),
                            stop=(u == 2),
                            perf_mode=DR,
                        )
                    nc.scalar.mul(out=v_sb[:, m, :], in_=pt, mul=1.0 / 32)

                # q32 = (32WqT).T @ x8, one m-tile -> q_sb[:, m, :] (carries x32)
                def qproj_m(q_sb, x8_sb, m):
                    pt = ps.tile([128, CH], F32, tag="ps")
                    for u in range(2):
                        nc.tensor.matmul(
                            pt,
                            wq_sb[:, 2 * u:2 * u + 2, bass.ts(m, 128)],
                            x8_sb[:, 2 * u:2 * u + 2, :],
                            start=(u == 0),
                            stop=(u == 1),
                            perf_mode=DR,
                        )
                    nc.vector.tensor_copy(out=q_sb[:, m, :], in_=pt)

                def load_chunk(c):
                    x8_sb = xp.tile([128, 4, CH], F8, tag="x8")
                    nc.gpsimd.dma_start(out=x8_sb, in_=x8_t[:, c, :, :])
                    xr_sb = xp.tile([128, 4, CH], BF16, tag="xr")
                    nc.gpsimd.dma_start(out=xr_sb, in_=xres_t[:, c, :, :])
                    return x8_sb, xr_sb

                # ---- main loop over pixel chunks (q software-pipelined) ----
                x8_cur, xr_cur = load_chunk(0)
                q_cur = qp.tile([128, 4, CH], BF16, name="q0")
                for m in range(4):
                    qproj_m(q_cur, x8_cur, m)

                for c in range(NCHUNK):
                    nxt = c + 1 < NCHUNK
                    if nxt:
                        x8_n, xr_n = load_chunk(c + 1)
                        q_n = qp.tile([128, 4, CH], BF16, name=f"q{c + 1}")
                    q_sb = q_cur

                    # per head-pair attention; next chunk's q m-tiles fill
                    # the exp-latency windows between pairs
                    oc_sb = ocp.tile([128, 4, CH], F8)
                    for p in range(4):  # head pair p -> heads 2p, 2p+1
                        # simT (x1024) for both heads, bf16 matmuls
                        pts = [ps2.tile([128, 2, CH], F32, tag="sim",
                                        name=f"psim{p}_{hh2}")
                               for hh2 in range(2)]
                        for hh in range(2):
                            for j in range(2):
                                h0 = hh * 64
                                nc.tensor.matmul(
                                    pts[hh][:, j, :],
                                    kT_sb[h0:h0 + 64, p, bass.ts(j, 128)],
                                    q_sb[h0:h0 + 64, p, :],
                                    start=True,
                                    stop=True,
                                )
                        # E = exp(simT32 * SCALE/1024) -> bf16, one ACT op/head
                        e_tiles = []
                        for hh in range(2):
                            e_sb = ep.tile([128, 2, CH], BF16, tag="e")
                            nc.scalar.activation(
                                out=e_sb,
                                in_=pts[hh],
                                func=mybir.ActivationFunctionType.Exp,
                                scale=float(SCALE) / 1024.0,
                            )
                            e_tiles.append(e_sb)

                        # attn@v + denominator, col-group pairs (bf16)
                        pav = ps.tile([128, CH], F32, tag="ps")
                        pS = ps.tile([128, CH], F32, tag="ps", name=f"pS{p}")
                        for kj in range(2):
                            for hh in range(2):
                                h = 2 * p + hh
                                h0 = hh * 64
                                nc.tensor.matmul(
                                    pav[h0:h0 + 64, :],
                                    v_sb[:, kj, bass.ds(h * 64, 64)],
                                    e_tiles[hh][:, kj, :],
                                    start=(kj == 0),
                                    stop=(kj == 1),
                                    skip_group_check=True,
                                )
                        for kj in range(2):
                            for hh in range(2):
                                h0 = hh * 64
                                nc.tensor.matmul(
                                    pS[h0:h0 + 64, :],
                                    ones_sb,
                                    e_tiles[hh][:, kj, :],
                                    start=(kj == 0),
                                    stop=(kj == 1),
                                    skip_group_check=True,
                                )
                        # normalize: oc = pav / pS  (full 128-width), fp8 out
                        r_sb = rp.tile([128, CH], F32, tag="r")
                        nc.vector.reciprocal_approx_fast(out=r_sb, in_=pS)
                        nc.vector.tensor_mul(out=oc_sb[:, p, :], in0=pav, in1=r_sb)

                        if nxt:
                            qproj_m(q_n, x8_n, p)

                    # out projection (x32) + residual 32*(x+bout), bf16 out
                    o_sb = outp.tile([128, 4, CH], BF16)
                    for m in range(4):
                        pt = ps.tile([128, CH], F32, tag="ps")
                        for u in range(2):
                            nc.tensor.matmul(
                                pt,
                                wo_sb[:, 2 * u:2 * u + 2, bass.ts(m, 128)],
                                oc_sb[:, 2 * u:2 * u + 2, :],
                                start=(u == 0),
                                stop=(u == 1),
                                perf_mode=DR,
                            )
                        nc.vector.tensor_add(
                            out=o_sb[:, m, :],
                            in0=pt,
                            in1=xr_cur[:, m, :],
                        )
                    nc.gpsimd.dma_start(out=out_t[:, c, :, :], in_=o_sb)
                    if nxt:
                        q_cur, xr_cur = q_n, xr_n

    nc.compile()
    return nc


_NC_CACHE = None


def _get_nc():
    global _NC_CACHE
    if _NC_CACHE is None:
        _NC_CACHE = build_bass()
    return _NC_CACHE


def _pshuf_w(a):
    """[T*128, E] -> [128, T, E] partition-major contiguous."""
    t = a.shape[0] // 128
    return np.ascontiguousarray(a.reshape(t, 128, a.shape[1]).transpose(1, 0, 2))


def _pshuf_x(a):
    """[512, 4096] -> [128, NCHUNK, 4, CH] partition-major, chunk-contiguous."""
    return np.ascontiguousarray(
        a.reshape(4, 128, NCHUNK, CH).transpose(1, 2, 0, 3))


def make_in_maps(x, context, Wq, Wkv, Wout, bout):
    """Host-side prep: shard over batch, pre-transpose weights, quantize fp8."""
    f = np.float32
    bf = ml_dtypes.bfloat16
    f8 = ml_dtypes.float8_e4m3
    wqT = _pshuf_w((Wq.T * np.float32(32)).astype(f8))
    wkT = _pshuf_w((Wkv[:512].T * np.float32(32)).astype(f8))
    wvT = _pshuf_w((Wkv[512:].T * np.float32(32)).astype(f8))
    woT = _pshuf_w((Wout.T * np.float32(32)).astype(f8))
    bout = np.asarray(bout, dtype=f)
    in_maps = []
    for b in range(B):
        xf = np.ascontiguousarray(x[b].reshape(DIM, HW), dtype=f)
        in_maps.append({
            "x8": _pshuf_x(xf.astype(f8)),
            "xres32": _pshuf_x(((xf + bout[:, None]) * np.float32(32)).astype(bf)),
            "ctxT8": _pshuf_w(np.ascontiguousarray(context[b].T).astype(f8)),
            "wqT8": wqT,
            "wkT8": wkT,
            "wvT8": wvT,
            "woT8": woT,
        })
    return in_maps


def postprocess_out(raw):
    """Device out [128, NCHUNK, 4, CH] bf16 carrying x32 -> [512, 4096] f32."""
    o = np.asarray(raw).astype(np.float32).reshape(128, NCHUNK, 4, CH)
    return (o / np.float32(32)).transpose(2, 0, 1, 3).reshape(DIM, HW)


def kernel(x, context, Wq, Wkv, Wout, bout):
    x = np.asarray(x)
    context = np.asarray(context)
    nc = _get_nc()
    in_maps = make_in_maps(x, context, np.asarray(Wq), np.asarray(Wkv),
                           np.asarray(Wout), np.asarray(bout))
    res = run_bass_kernel_spmd(nc, in_maps, core_ids=list(range(B)))
    outs = [postprocess_out(res.results[b]["out"]) for b in range(B)]
    return np.stack(outs, axis=0).reshape(B, DIM, 64, 64)
